# revision 1
# baseline (speedup 1.0000x reference)
"""2-layer GCN (PyG GCNConv x2) on 8 Trainium2 NeuronCores via Bass/Tile.

Sharding (per hint): nodes split contiguously across 8 cores (x rows, degree,
output); edges partitioned by destination core so the segment-sum is local;
weight matrices replicated.

Per core:
  1) transform-first: hT = W1^T @ xT chunks on PE, rows scaled by deg^-1/2
     (folds dinv[src] into the table), written node-major to a DRAM shard.
  2) AllGather the 64-ch table across the 8 cores.
  3) local segment-sum: edges (+self-loops) sorted by (table-chunk, dst
     block, dst). The int16-indexed dma_gather instruction fetches messages
     in compact 128-edge groups ([128, Q, 64] tiles); the table is processed
     in <=32768-row chunks (int16 index range). Per group, a one-hot
     selection matrix (DVE is_equal of a constant iota against the group's
     dst-slot column) is matmul-accumulated into PSUM: ps[slot, ch] +=
     sum_e onehot[e, slot] * msg[e, ch] — the segment-sum for one 128-dst
     block. Pad edges point at a guaranteed-zero pad row, contributing 0.
     Epilogue: x dinv[dst] + bias (+relu for layer 1).
  4) layer 2 repeats with W2/b2, reusing the same edge structure, after a
     second AllGather of g' = dinv * (h2 @ W2).

Host does only sharding-related prep (bucketing edges by dst core, degree
compute, degree-sorting nodes per core, building index/slot streams). All
FLOPs and heavy data movement run on the NeuronCores.
"""

import numpy as np

P = 128
N_CORES = 8
HID = 64
CH_MAX = 32768   # int16 index range per dma_gather call
QG = 80          # max 128-edge groups per gather call (tile [128, QG, 64])


# ----------------------------------------------------------------- host prep
def _preprocess(x, edge_index, n_cores=N_CORES):
    n_nodes = x.shape[0]
    npc = n_nodes // n_cores
    assert npc * n_cores == n_nodes
    nb = -(-npc // P)
    shard = nb * P
    assert shard > npc, "need pad rows in each shard (zero rows for padding)"
    total = n_cores * shard
    n_chunks = -(-total // CH_MAX)
    ch = -(-total // n_chunks // P) * P
    chunk_bases = [c * ch for c in range(n_chunks)]
    chunk_rows = [min(ch, total - c * ch) for c in range(n_chunks)]

    pad_rows = np.concatenate(
        [k * shard + np.arange(npc, shard) for k in range(n_cores)])
    zrow = []
    for c in range(n_chunks):
        inr = pad_rows[(pad_rows >= chunk_bases[c])
                       & (pad_rows < chunk_bases[c] + chunk_rows[c])]
        assert len(inr), f"no zero row in chunk {c}"
        zrow.append(int(inr[0]))

    src = np.asarray(edge_index[0], dtype=np.int64)
    dst = np.asarray(edge_index[1], dtype=np.int64)
    deg = np.bincount(dst, minlength=n_nodes).astype(np.int64) + 1
    dinv = (1.0 / np.sqrt(deg.astype(np.float64))).astype(np.float32)

    loop = np.arange(n_nodes, dtype=np.int64)
    src_all = np.concatenate([src, loop])
    dst_all = np.concatenate([dst, loop])

    sorted_nodes = []
    row_of = np.empty(n_nodes, dtype=np.int64)
    pos_of = np.empty(n_nodes, dtype=np.int64)
    for k in range(n_cores):
        nodes_k = np.arange(k * npc, (k + 1) * npc)
        order = np.argsort(-deg[nodes_k], kind="stable")
        sn = nodes_k[order]
        sorted_nodes.append(sn)
        row_of[sn] = k * shard + np.arange(npc)
        pos_of[sn] = np.arange(npc)

    # per-core edges keyed by (chunk, dst position); per-(b, c) counts
    e_owner = dst_all // npc
    per_core = []
    cnt_bc = np.zeros((n_cores, nb, n_chunks), dtype=np.int64)
    for k in range(n_cores):
        m = e_owner == k
        p_e = pos_of[dst_all[m]]
        sr_e = row_of[src_all[m]]
        c_e = sr_e // ch
        key = c_e * shard + p_e            # chunk-major, then dst position
        perm = np.argsort(key, kind="stable")
        p_s, c_s, sr_s = p_e[perm], c_e[perm], sr_e[perm]
        b_s = p_s // P
        cb = np.bincount(c_s * nb + b_s, minlength=n_chunks * nb)
        cnt_bc[k] = cb.reshape(n_chunks, nb).T
        per_core.append((p_s, c_s, sr_s))

    # global group counts per (block, chunk)
    G = np.zeros((nb, n_chunks), dtype=np.int64)
    for b in range(nb):
        for c in range(n_chunks):
            mx = int(cnt_bc[:, b, c].max())
            G[b, c] = -(-mx // P) if mx else 0
    n_pos = int(P * G.sum())

    first_c, last_c = {}, {}
    for b in range(nb):
        nz = np.nonzero(G[b])[0]
        assert len(nz) > 0
        first_c[b] = int(nz[0])
        last_c[b] = int(nz[-1])

    # call plan: chunk-major, pack (b, c) runs into calls of <= QG groups
    calls = []
    for c in range(n_chunks):
        cur, lo = [], 0
        for b in range(nb):
            g = int(G[b, c])
            if g == 0:
                continue
            assert g <= QG, f"G[{b},{c}]={g} exceeds QG"
            if lo + g > QG:
                calls.append((c, cur))
                cur, lo = [], 0
            cur.append((b, lo, g))
            lo += g
        if cur:
            calls.append((c, cur))

    # run base offsets (in edges) inside the global padded stream
    run_base = np.zeros((nb, n_chunks), dtype=np.int64)
    pos = 0
    for c, blocks in calls:
        for b, lo, g in blocks:
            run_base[b, c] = pos
            pos += g * P
    assert pos == n_pos

    gidx, slots = [], []
    for k in range(n_cores):
        p_s, c_s, sr_s = per_core[k]
        b_s = p_s // P
        # rank of each edge within its (b, c) run
        key2 = c_s * nb + b_s
        cb = np.bincount(key2, minlength=n_chunks * nb)
        starts = np.zeros(n_chunks * nb, dtype=np.int64)
        starts[1:] = np.cumsum(cb)[:-1]
        j = np.arange(len(p_s)) - starts[key2]
        flat_pos = run_base[b_s, c_s] + j
        zr = np.asarray(zrow, dtype=np.int64) - np.asarray(chunk_bases,
                                                           dtype=np.int64)
        idx_flat = np.empty(n_pos, dtype=np.int16)
        slot_flat = np.zeros(n_pos, dtype=np.float32)
        # fill pads per chunk with that chunk's zero row
        for c, blocks in calls:
            for b, lo, g in blocks:
                rb = run_base[b, c]
                idx_flat[rb:rb + g * P] = zr[c]
        cbase = np.asarray(chunk_bases, dtype=np.int64)
        idx_flat[flat_pos] = (sr_s - cbase[c_s]).astype(np.int16)
        slot_flat[flat_pos] = (p_s % P).astype(np.float32)
        # wrapped int16 stream: per call [128, 8*Q]
        segs = []
        for c, blocks in calls:
            qn = sum(g for _, _, g in blocks)
            rb = run_base[blocks[0][0], c]
            fl = idx_flat[rb:rb + qn * P]
            segs.append(np.tile(fl.reshape(-1, 16).T, (8, 1)))
        gidx.append(np.ascontiguousarray(np.concatenate(segs, axis=1)))
        # slot columns [128, n_groups]
        slots.append(np.ascontiguousarray(
            slot_flat.reshape(-1, P).T))

    dinv_cols, xts = [], []
    for k in range(n_cores):
        tmp = np.zeros(shard, dtype=np.float32)
        tmp[:npc] = dinv[sorted_nodes[k]]
        dinv_cols.append(np.ascontiguousarray(tmp.reshape(nb, P).T))
        xt = np.zeros((x.shape[1], shard), dtype=np.float32)
        xt[:, :npc] = np.asarray(x, dtype=np.float32)[sorted_nodes[k]].T
        xts.append(xt)

    iota = np.tile(np.arange(P, dtype=np.float32)[None, :], (P, 1))

    return dict(
        n_nodes=n_nodes, npc=npc, nb=nb, shard=shard, n_chunks=n_chunks,
        chunk_bases=chunk_bases, chunk_rows=chunk_rows, calls=calls,
        first_c=first_c, last_c=last_c, wtot=gidx[0].shape[1],
        n_groups=int(G.sum()), n_pos=n_pos, iota=iota,
        gidx=gidx, slots=slots, dinv_cols=dinv_cols, xts=xts,
        sorted_nodes=sorted_nodes,
    )


# ------------------------------------------------------------- bass program
def _build(in_ch, meta, n_cores=N_CORES, stage=4):
    import concourse.bacc as bacc
    import concourse.tile as tile
    from concourse import mybir
    from concourse.masks import make_identity

    f32 = mybir.dt.float32
    i16 = mybir.dt.int16
    kc = in_ch // P
    rg = [list(range(n_cores))]
    shard, nb = meta["shard"], meta["nb"]
    calls, first_c, last_c = meta["calls"], meta["first_c"], meta["last_c"]
    chunk_bases, chunk_rows = meta["chunk_bases"], meta["chunk_rows"]
    wtot, n_groups = meta["wtot"], meta["n_groups"]

    nc = bacc.Bacc("TRN2", target_bir_lowering=False, debug=False,
                   num_devices=n_cores)
    xT_d = nc.dram_tensor("xT", [in_ch, shard], f32, kind="ExternalInput").ap()
    gidx_d = nc.dram_tensor("gidx", [P, wtot], i16, kind="ExternalInput").ap()
    slot_d = nc.dram_tensor("slots", [P, n_groups], f32,
                            kind="ExternalInput").ap()
    iota_d = nc.dram_tensor("iota", [P, P], f32, kind="ExternalInput").ap()
    dinv_d = nc.dram_tensor("dinv", [P, nb], f32, kind="ExternalInput").ap()
    w1_d = nc.dram_tensor("W1", [in_ch, HID], f32, kind="ExternalInput").ap()
    b1_d = nc.dram_tensor("b1", [1, HID], f32, kind="ExternalInput").ap()
    w2_d = nc.dram_tensor("W2", [HID, HID], f32, kind="ExternalInput").ap()
    b2_d = nc.dram_tensor("b2", [1, HID], f32, kind="ExternalInput").ap()
    out_d = nc.dram_tensor("out", [shard, HID], f32, kind="ExternalOutput").ap()

    with tile.TileContext(nc) as tc:
        with tc.tile_pool(name="const", bufs=1) as cp, \
             tc.tile_pool(name="sb", bufs=3) as sb, \
             tc.tile_pool(name="red", bufs=4) as rp, \
             tc.tile_pool(name="oh", bufs=4) as ohp, \
             tc.tile_pool(name="gat", bufs=2) as gp, \
             tc.tile_pool(name="accp", bufs=1) as ap_, \
             tc.tile_pool(name="ps", bufs=8, space="PSUM") as pp, \
             tc.tile_pool(name="dram", bufs=1, space="DRAM") as dp:

            w1 = cp.tile([P, kc, HID], f32)
            nc.sync.dma_start(w1[:], w1_d.rearrange("(c p) h -> p c h", p=P))
            w2 = cp.tile([HID, HID], f32)
            nc.sync.dma_start(w2[:], w2_d[:])
            ident = cp.tile([P, P], f32)
            make_identity(nc, ident[:])
            iota_sb = cp.tile([P, P], f32)
            nc.sync.dma_start(iota_sb[:], iota_d[:])
            dinv_sb = cp.tile([P, nb], f32)
            nc.sync.dma_start(dinv_sb[:], dinv_d[:])
            b1_row = cp.tile([1, HID], f32)
            nc.sync.dma_start(b1_row[:], b1_d[:])
            b1_bc = cp.tile([P, HID], f32)
            nc.gpsimd.partition_broadcast(b1_bc[:], b1_row[:])
            b2_row = cp.tile([1, HID], f32)
            nc.sync.dma_start(b2_row[:], b2_d[:])
            b2_bc = cp.tile([P, HID], f32)
            nc.gpsimd.partition_broadcast(b2_bc[:], b2_row[:])

            shard1 = dp.tile([shard, HID], f32)
            table1 = dp.tile([n_cores * shard, HID], f32)
            shard2 = dp.tile([shard, HID], f32)
            table2 = dp.tile([n_cores * shard, HID], f32)

            # ---- layer-1 transform: h1' = dinv * (x @ W1), node-major.
            # Two node-blocks per chain (N=256 matmuls) to halve PE
            # instruction count and deepen pipelining.
            xT_r = xT_d.rearrange("(c p) n -> p c n", p=P)
            pairs = [(t, min(t + 2, nb)) for t in range(0, nb, 2)]
            for t0, t1 in pairs:
                w = (t1 - t0) * P
                xt = sb.tile([P, kc, 2 * P], f32, tag="xt")
                nc.sync.dma_start(xt[:, :, :w],
                                  xT_r[:, :, t0 * P:t0 * P + w])
                hT = pp.tile([HID, 2 * P], f32, tag="ps")
                for c in range(kc):
                    nc.tensor.matmul(out=hT[:, :w], lhsT=w1[:, c, :],
                                     rhs=xt[:, c, :w],
                                     start=(c == 0), stop=(c == kc - 1))
                hTs = sb.tile([HID, 2 * P], f32, tag="hTs")
                nc.scalar.copy(hTs[:, :w], hT[:, :w])
                for t in range(t0, t1):
                    off = (t - t0) * P
                    h = pp.tile([P, HID], f32, tag="ps")
                    nc.tensor.transpose(out=h[:], in_=hTs[:, off:off + P],
                                        identity=ident[:HID, :HID])
                    hp = sb.tile([P, HID], f32, tag="hp")
                    nc.scalar.mul(hp[:], h[:], mul=dinv_sb[:, t:t + 1])
                    nc.sync.dma_start(shard1[t * P:(t + 1) * P, :], hp[:])

            if stage != 0:
                nc.gpsimd.collective_compute(
                    "AllGather", mybir.AluOpType.bypass, replica_groups=rg,
                    ins=[shard1.opt()], outs=[table1.opt()])

            if stage == 0:
                for b in range(nb):
                    t0 = sb.tile([P, HID], f32, tag="cp")
                    nc.sync.dma_start(t0[:], shard1[b * P:(b + 1) * P, :])
                    nc.sync.dma_start(out_d[b * P:(b + 1) * P, :], t0[:])
            if stage == 1:
                for b in range(nb):
                    t0 = sb.tile([P, HID], f32, tag="cp")
                    nc.sync.dma_start(t0[:], table1[b * P:(b + 1) * P, :])
                    nc.sync.dma_start(out_d[b * P:(b + 1) * P, :], t0[:])

            def aggregate(table, layer, gather_only=False, finish=None):
                colpos = 0
                gpos = 0
                acc = ap_.tile([P, nb * HID], f32, tag=f"acc{layer}",
                               name=f"acc{layer}")
                done = []
                for c, blocks in calls:
                    qn = sum(g for _, _, g in blocks)
                    gidx_sb = sb.tile([P, 8 * qn], i16, tag="gidx")
                    nc.sync.dma_start(
                        gidx_sb[:], gidx_d[:, colpos:colpos + 8 * qn])
                    colpos += 8 * qn
                    slots_sb = sb.tile([P, qn], f32, tag="slots")
                    nc.sync.dma_start(slots_sb[:], slot_d[:, gpos:gpos + qn])
                    gt = gp.tile([P, qn, HID], f32, tag="g")
                    nc.gpsimd.dma_gather(
                        out_ap=gt[:],
                        in_ap=table[chunk_bases[c]:
                                    chunk_bases[c] + chunk_rows[c], :],
                        idxs_ap=gidx_sb[:], num_idxs=P * qn,
                        num_idxs_reg=P * qn, elem_size=HID,
                        single_packet=False)
                    if gather_only:
                        gpos += qn
                        continue
                    for b, lo, g in blocks:
                        ps = pp.tile([P, HID], f32, tag="ps")
                        oh = ohp.tile([P, g, P], f32, tag="oh")
                        nc.vector.tensor_tensor(
                            out=oh[:],
                            in0=iota_sb[:].rearrange("p (g j) -> p g j", g=1)
                                .to_broadcast([P, g, P]),
                            in1=slots_sb[:, lo:lo + g]
                                .rearrange("p (g j) -> p g j", j=1)
                                .to_broadcast([P, g, P]),
                            op=mybir.AluOpType.is_equal)
                        for q in range(lo, lo + g):
                            nc.tensor.matmul(out=ps[:], lhsT=oh[:, q - lo, :],
                                             rhs=gt[:, q, :],
                                             start=(q == lo),
                                             stop=(q == lo + g - 1))
                        a_sl = acc[:, b * HID:(b + 1) * HID]
                        if c == first_c[b]:
                            nc.scalar.copy(a_sl, ps[:])
                        else:
                            nc.vector.tensor_add(out=a_sl, in0=a_sl,
                                                 in1=ps[:])
                        if c == last_c[b]:
                            if finish is not None:
                                finish(b, a_sl)
                            else:
                                done.append((b, a_sl))
                    gpos += qn
                return done

            # ---- layer-1 aggregation + layer-2 transform (inline finish)
            def finish1(b, a_sl):
                accb = rp.tile([P, HID], f32, tag="accb", name="accb")
                nc.vector.tensor_scalar(
                    out=accb[:], in0=a_sl, scalar1=dinv_sb[:, b:b + 1],
                    scalar2=None, op0=mybir.AluOpType.mult)
                acc2 = rp.tile([P, HID], f32, tag="acc2", name="acc2")
                nc.vector.tensor_add(out=acc2[:], in0=accb[:], in1=b1_bc[:])
                h2 = sb.tile([P, HID], f32, tag="h2", bufs=6, name="h2")
                nc.scalar.activation(h2[:], acc2[:],
                                     mybir.ActivationFunctionType.Relu)
                h2T = pp.tile([HID, P], f32, tag="ps", name="h2T")
                nc.tensor.transpose(out=h2T[:], in_=h2[:], identity=ident[:])
                h2Ts = sb.tile([HID, P], f32, tag="h2Ts", bufs=6, name="h2Ts")
                nc.scalar.copy(h2Ts[:], h2T[:])
                gT = pp.tile([HID, P], f32, tag="ps", name="gT")
                nc.tensor.matmul(out=gT[:], lhsT=w2[:], rhs=h2Ts[:],
                                 start=True, stop=True)
                gTs = sb.tile([HID, P], f32, tag="gTs", bufs=6, name="gTs")
                nc.scalar.copy(gTs[:], gT[:])
                gg = pp.tile([P, HID], f32, tag="ps", name="gg")
                nc.tensor.transpose(out=gg[:], in_=gTs[:],
                                    identity=ident[:HID, :HID])
                gsb = sb.tile([P, HID], f32, tag="gsb", bufs=6, name="gsb")
                nc.scalar.mul(gsb[:], gg[:], mul=dinv_sb[:, b:b + 1])
                nc.sync.dma_start(shard2[b * P:(b + 1) * P, :], gsb[:])

            def finish3(b, a_sl):
                ob = rp.tile([P, HID], f32, tag="ob", name="ob")
                nc.vector.tensor_scalar(
                    out=ob[:], in0=a_sl, scalar1=dinv_sb[:, b:b + 1],
                    scalar2=None, op0=mybir.AluOpType.mult)
                nc.sync.dma_start(out_d[b * P:(b + 1) * P, :], ob[:])

            if stage == 2:
                aggregate(table1, 1, gather_only=True)
            elif stage == 3:
                aggregate(table1, 1, finish=finish3)
            elif stage >= 4:
                aggregate(table1, 1, finish=finish1)

            if stage >= 4:
                nc.gpsimd.collective_compute(
                    "AllGather", mybir.AluOpType.bypass, replica_groups=rg,
                    ins=[shard2.opt()], outs=[table2.opt()])

            # ---- layer-2 aggregation (inline final epilogue)
            def finish2(b, a_sl):
                acc4 = rp.tile([P, HID], f32, tag="acc4", name="acc4")
                nc.vector.tensor_scalar(
                    out=acc4[:], in0=a_sl, scalar1=dinv_sb[:, b:b + 1],
                    scalar2=None, op0=mybir.AluOpType.mult)
                osb = sb.tile([P, HID], f32, tag="osb", name="osb")
                nc.vector.tensor_add(out=osb[:], in0=acc4[:], in1=b2_bc[:])
                nc.sync.dma_start(out_d[b * P:(b + 1) * P, :], osb[:])

            if stage >= 4:
                aggregate(table2, 2, finish=finish2)

    nc.compile()
    return nc


# ------------------------------------------------------------------- driver
_CACHE = {}


def _get_nc(in_ch, meta):
    key = (in_ch, meta["shard"], meta["wtot"],
           tuple((c, tuple(b)) for c, bl in meta["calls"] for b in bl))
    if key not in _CACHE:
        _CACHE[key] = _build(in_ch, meta)
    return _CACHE[key]


def _in_maps(pre, W1, b1, W2, b2):
    maps = []
    for k in range(N_CORES):
        maps.append({
            "xT": pre["xts"][k],
            "gidx": pre["gidx"][k],
            "slots": pre["slots"][k],
            "iota": pre["iota"],
            "dinv": pre["dinv_cols"][k],
            "W1": W1, "b1": b1, "W2": W2, "b2": b2,
        })
    return maps


def kernel(x, edge_index, W1, b1, W2, b2):
    from concourse.bass_utils import run_bass_kernel_spmd

    x = np.asarray(x, dtype=np.float32)
    W1 = np.ascontiguousarray(np.asarray(W1, dtype=np.float32))
    W2 = np.ascontiguousarray(np.asarray(W2, dtype=np.float32))
    b1 = np.asarray(b1, dtype=np.float32).reshape(1, HID)
    b2 = np.asarray(b2, dtype=np.float32).reshape(1, HID)

    pre = _preprocess(x, edge_index)
    nc = _get_nc(x.shape[1], pre)
    res = run_bass_kernel_spmd(nc, _in_maps(pre, W1, b1, W2, b2),
                               core_ids=list(range(N_CORES)))

    npc = pre["npc"]
    out = np.empty((pre["n_nodes"], HID), dtype=np.float32)
    for k in range(N_CORES):
        out[pre["sorted_nodes"][k]] = res.results[k]["out"][:npc]
    return out



# revision 8
# speedup vs baseline: 34.4488x; 34.4488x over previous
"""2-layer GCN (PyG GCNConv x2) on 8 Trainium2 NeuronCores via Bass/Tile. v4.

Sharding: nodes split contiguously across 8 cores (x rows, degree, output);
edges partitioned by destination core so the segment-sum is local; weights
replicated.

v4 vs v3: pipelined collectives.
  * block-cyclic table layout: global row = (pos//pp)*ch + core*pp + pos%pp
    with pp = shard/n_chunks, so table chunk c is the AllGather of each
    core's piece c. Each layer issues n_chunks small AllGathers instead of
    one big one; gather calls on chunk c only wait for AllGather c.
  * piece-major aggregation: dst blocks are processed in n_chunks pieces;
    after piece q of layer-1 finishes, its shard2 piece AllGather is issued
    immediately, overlapping the remaining layer-1 work and letting layer-2
    gathers start while layer 1 is still draining.
  * pads point at row 0 with slot -1 (one-hot selects nothing), so no
    zero-row machinery is needed.

v3 vs v2:
  * dense edge stream per (core, chunk): 128-edge wraps may span dst-block
    boundaries (one-hot slot columns are -1 outside the pair's block), so the
    only gather padding is at chunk tails -> ~28% fewer gather descriptors.
  * self-loop messages are not gathered; the local shard block is added in
    the epilogue instead (saves another ~6% of descriptors).
  * dma_gather calls round-robin over 4 SWDGE queues (parallel Q7
    descriptor generation pipelines), deep gather-tile ring to keep all
    queues busy.

Per core:
  1) transform: hT = W1^T @ xT on PE, rows scaled by deg^-1/2, written
     node-major to a DRAM shard.
  2) AllGather the 64-ch table across the 8 cores.
  3) local segment-sum: real edges sorted by (src-chunk, dst pos). Each
     dma_gather call fetches <=QG 128-edge wraps ([128, qn, 64] tile) from
     one <=32768-row table chunk (int16 index range). Per (wrap, dst block)
     pair, a one-hot selection matrix (DVE is_equal of iota vs the pair's
     slot column; -1 entries select nothing) is matmul-accumulated into
     PSUM: ps_b[slot, ch] += sum_e onehot[e, slot] * msg[e, ch]. Chunk-tail
     pad edges point at a guaranteed-zero pad row.
     Epilogue per block: + local shard block (self-loop), x dinv[dst],
     + bias (+relu, + W2 transform for layer 1).
  4) layer 2 repeats with W2/b2 on a second AllGather'd table.

Host does only sharding-related prep (bucketing/sorting edges, degree
compute, building index/slot streams). All FLOPs and heavy data movement
run on the NeuronCores.
"""

import numpy as np

P = 128
N_CORES = 8
HID = 64
CH_MAX = 32768   # int16 index range per dma_gather call
QG = 64          # max 128-edge wraps per gather call (tile [128, QG, 64])
N_QUEUES = 4     # SWDGE queues; gathers round-robin across Q7 cpu pairs
GBUFS = 5        # gather-tile ring depth (> N_QUEUES keeps all queues busy)
OHB = 8          # one-hot pairs built per DVE op


# ----------------------------------------------------------------- host prep
def _preprocess(x, edge_index, n_cores=N_CORES):
    n_nodes = x.shape[0]
    npc = n_nodes // n_cores
    assert npc * n_cores == n_nodes
    nb = -(-npc // P)
    shard = nb * P
    assert shard > npc, "need pad rows in each shard"
    total = n_cores * shard
    n_chunks = -(-total // CH_MAX)
    assert shard % n_chunks == 0
    pp = shard // n_chunks           # rows per (core, piece)
    ch = n_cores * pp                # table rows per chunk
    chunk_bases = [c * ch for c in range(n_chunks)]
    chunk_rows = [ch] * n_chunks

    src = np.asarray(edge_index[0], dtype=np.int64)
    dst = np.asarray(edge_index[1], dtype=np.int64)
    deg = np.bincount(dst, minlength=n_nodes).astype(np.int64) + 1
    dinv = (1.0 / np.sqrt(deg.astype(np.float64))).astype(np.float32)

    sorted_nodes = []
    row_of = np.empty(n_nodes, dtype=np.int64)
    pos_of = np.empty(n_nodes, dtype=np.int64)
    for k in range(n_cores):
        nodes_k = np.arange(k * npc, (k + 1) * npc)
        order = np.argsort(-deg[nodes_k], kind="stable")
        sn = nodes_k[order]
        sorted_nodes.append(sn)
        pos = np.arange(npc)
        row_of[sn] = (pos // pp) * ch + k * pp + pos % pp
        pos_of[sn] = pos

    # per-core real edges sorted by (src chunk, dst pos); no self-loops
    e_owner = dst // npc
    per_core = []
    E_kc = np.zeros((n_cores, n_chunks), dtype=np.int64)
    for k in range(n_cores):
        m = e_owner == k
        p_e = pos_of[dst[m]]
        sr_e = row_of[src[m]]
        c_e = sr_e // ch
        perm = np.argsort(c_e * shard + p_e, kind="stable")
        p_s, c_s, sr_s = p_e[perm], c_e[perm], sr_e[perm]
        E_kc[k] = np.bincount(c_s, minlength=n_chunks)
        per_core.append((p_s, c_s, sr_s))

    Gc = [int(-(-E_kc[:, c].max() // P)) for c in range(n_chunks)]
    n_pos = int(P * sum(Gc))
    wrap_base = np.concatenate([[0], np.cumsum(Gc)])  # wraps before chunk c

    # (wrap, block) pair union across cores, per chunk
    pair_keys = [set() for _ in range(n_chunks)]
    for k in range(n_cores):
        p_s, c_s, _ = per_core[k]
        for c in range(n_chunks):
            pc = p_s[c_s == c]
            if len(pc) == 0:
                continue
            w_s = np.arange(len(pc)) // P
            b_s = pc // P
            pair_keys[c].update(np.unique(w_s * nb + b_s).tolist())

    # pair index map per chunk: (w, b) -> global pair column j, in program
    # emission order: piece-major (q), then chunk (c), then w-range, then b.
    # A wrap belongs to the piece of its smallest block, so wrap ranges per
    # (q, c) are contiguous.
    chunk_keys = []
    wrap_piece = []
    for c in range(n_chunks):
        keys = np.array(sorted(pair_keys[c]), dtype=np.int64)
        w_all, b_all = keys // nb, keys % nb
        chunk_keys.append((w_all, b_all))
        min_b = np.full(Gc[c], nb, dtype=np.int64)
        np.minimum.at(min_b, w_all, b_all)
        assert (min_b < nb).all(), "empty wrap"
        wrap_piece.append((min_b * P) // pp)

    calls = []       # (c, w0, qn, blocks) ; blocks=[(b, [(w_loc, j)...])...]
    pair_map = [np.full((Gc[c], nb), -1, dtype=np.int64)
                for c in range(n_chunks)]
    occ = {}         # b -> list of (call index, block record)
    piece_call_end = []   # call count after each piece's layer-1 calls
    j = 0
    for q in range(n_chunks):
        for c in range(n_chunks):
            w_all, b_all = chunk_keys[c]
            wsel = np.nonzero(wrap_piece[c] == q)[0]
            if len(wsel) == 0:
                continue
            wlo, whi = int(wsel[0]), int(wsel[-1]) + 1
            for w0 in range(wlo, whi, QG):
                qn = min(QG, whi - w0)
                m = (w_all >= w0) & (w_all < w0 + qn)
                blocks = []
                for b in np.unique(b_all[m]):
                    ws = w_all[m][b_all[m] == b]
                    wj = []
                    for w in ws:
                        pair_map[c][w, b] = j
                        wj.append((int(w - w0), j))
                        j += 1
                    blocks.append([int(b), wj, None, False])
                ci = len(calls)
                for blk in blocks:
                    occ.setdefault(blk[0], []).append((ci, blk))
                calls.append((c, w0, qn, blocks))
        piece_call_end.append(len(calls))
    n_pairs = j

    # acc op (copy on first call-occurrence, add after) + finish flag
    for b, lst in occ.items():
        for i, (ci, blk) in enumerate(lst):
            blk[2] = "copy" if i == 0 else "add"
        lst[-1][1][3] = True
    assert sorted(occ.keys()) == list(range(nb))

    # per-core slot columns and gather index stream
    gidx, slots = [], []
    for k in range(n_cores):
        p_s, c_s, sr_s = per_core[k]
        slots_k = np.full((n_pairs, P), -1.0, dtype=np.float32)
        idx_flat = np.zeros(n_pos, dtype=np.int16)   # pads: row 0, slot -1
        for c in range(n_chunks):
            sel = c_s == c
            pc, sr = p_s[sel], sr_s[sel]
            base = int(wrap_base[c]) * P
            pos = np.arange(len(pc))
            idx_flat[base + pos] = (sr - chunk_bases[c]).astype(np.int16)
            w_s, r_s, b_s = pos // P, pos % P, pc // P
            j_s = pair_map[c][w_s, b_s]
            assert (j_s >= 0).all()
            slots_k[j_s, r_s] = (pc - b_s * P).astype(np.float32)
        segs = []
        for c, w0, qn, _ in calls:
            fl = idx_flat[(wrap_base[c] + w0) * P:
                          (wrap_base[c] + w0 + qn) * P]
            segs.append(np.tile(fl.reshape(-1, 16).T, (8, 1)))
        gidx.append(np.ascontiguousarray(np.concatenate(segs, axis=1)))
        slots.append(np.ascontiguousarray(slots_k.T))

    dinv_cols, xts = [], []
    for k in range(n_cores):
        tmp = np.zeros(shard, dtype=np.float32)
        tmp[:npc] = dinv[sorted_nodes[k]]
        dinv_cols.append(np.ascontiguousarray(tmp.reshape(nb, P).T))
        xt = np.zeros((x.shape[1], shard), dtype=np.float32)
        xt[:, :npc] = np.asarray(x, dtype=np.float32)[sorted_nodes[k]].T
        xts.append(xt)

    iota = np.tile(np.arange(P, dtype=np.float32)[None, :], (P, 1))

    return dict(
        n_nodes=n_nodes, npc=npc, nb=nb, shard=shard, n_chunks=n_chunks,
        pp=pp, chunk_bases=chunk_bases, chunk_rows=chunk_rows, calls=calls,
        piece_call_end=piece_call_end, wtot=gidx[0].shape[1],
        n_pairs=n_pairs, n_pos=n_pos, iota=iota,
        gidx=gidx, slots=slots, dinv_cols=dinv_cols, xts=xts,
        sorted_nodes=sorted_nodes,
    )


# ------------------------------------------------------------- bass program
def _build(in_ch, meta, n_cores=N_CORES, stage=4):
    import concourse.bacc as bacc
    import concourse.tile as tile
    from concourse import mybir
    from concourse.masks import make_identity

    f32 = mybir.dt.float32
    i16 = mybir.dt.int16
    kc = in_ch // P
    rg = [list(range(n_cores))]
    shard, nb = meta["shard"], meta["nb"]
    calls = meta["calls"]
    chunk_bases, chunk_rows = meta["chunk_bases"], meta["chunk_rows"]
    wtot, n_pairs = meta["wtot"], meta["n_pairs"]
    n_chunks, pp = meta["n_chunks"], meta["pp"]
    piece_call_end = meta["piece_call_end"]
    ch = n_cores * pp

    nc = bacc.Bacc("TRN2", target_bir_lowering=False, debug=False,
                   num_devices=n_cores, num_swdge_queues=N_QUEUES)
    xT_d = nc.dram_tensor("xT", [in_ch, shard], f32, kind="ExternalInput").ap()
    gidx_d = nc.dram_tensor("gidx", [P, wtot], i16, kind="ExternalInput").ap()
    slot_d = nc.dram_tensor("slots", [P, n_pairs], f32,
                            kind="ExternalInput").ap()
    iota_d = nc.dram_tensor("iota", [P, P], f32, kind="ExternalInput").ap()
    dinv_d = nc.dram_tensor("dinv", [P, nb], f32, kind="ExternalInput").ap()
    w1_d = nc.dram_tensor("W1", [in_ch, HID], f32, kind="ExternalInput").ap()
    b1_d = nc.dram_tensor("b1", [1, HID], f32, kind="ExternalInput").ap()
    w2_d = nc.dram_tensor("W2", [HID, HID], f32, kind="ExternalInput").ap()
    b2_d = nc.dram_tensor("b2", [1, HID], f32, kind="ExternalInput").ap()
    out_d = nc.dram_tensor("out", [shard, HID], f32, kind="ExternalOutput").ap()

    with tile.TileContext(nc) as tc:
        with tc.tile_pool(name="const", bufs=1) as cp, \
             tc.tile_pool(name="sb", bufs=3) as sb, \
             tc.tile_pool(name="red", bufs=4) as rp, \
             tc.tile_pool(name="oh", bufs=4) as ohp, \
             tc.tile_pool(name="gat", bufs=GBUFS) as gp, \
             tc.tile_pool(name="gx", bufs=GBUFS) as gxp, \
             tc.tile_pool(name="accp", bufs=1) as ap_, \
             tc.tile_pool(name="ps", bufs=8, space="PSUM") as pp_, \
             tc.tile_pool(name="dram", bufs=1, space="DRAM") as dp:

            w1 = cp.tile([P, kc, HID], f32)
            nc.sync.dma_start(w1[:], w1_d.rearrange("(c p) h -> p c h", p=P))
            w2 = cp.tile([HID, HID], f32)
            nc.sync.dma_start(w2[:], w2_d[:])
            ident = cp.tile([P, P], f32)
            make_identity(nc, ident[:])
            iota_sb = cp.tile([P, P], f32)
            nc.sync.dma_start(iota_sb[:], iota_d[:])
            dinv_sb = cp.tile([P, nb], f32)
            nc.sync.dma_start(dinv_sb[:], dinv_d[:])
            b1_row = cp.tile([1, HID], f32)
            nc.sync.dma_start(b1_row[:], b1_d[:])
            b1_bc = cp.tile([P, HID], f32)
            nc.gpsimd.partition_broadcast(b1_bc[:], b1_row[:])
            b2_row = cp.tile([1, HID], f32)
            nc.sync.dma_start(b2_row[:], b2_d[:])
            b2_bc = cp.tile([P, HID], f32)
            nc.gpsimd.partition_broadcast(b2_bc[:], b2_row[:])

            shard1 = dp.tile([shard, HID], f32)
            table1 = dp.tile([n_cores * shard, HID], f32)
            shard2 = dp.tile([shard, HID], f32)
            table2 = dp.tile([n_cores * shard, HID], f32)

            def ag_piece(q, shard_t, table_t):
                nc.gpsimd.collective_compute(
                    "AllGather", mybir.AluOpType.bypass, replica_groups=rg,
                    ins=[shard_t[q * pp:(q + 1) * pp, :]],
                    outs=[table_t[q * ch:(q + 1) * ch, :]])

            # piece q of the shard is complete once the block holding its
            # last row has been written
            ag_after_block = [-(-pp * (q + 1) // P) - 1
                              for q in range(n_chunks)]

            # ---- layer-1 transform: h1' = dinv * (x @ W1), node-major.
            xT_r = xT_d.rearrange("(c p) n -> p c n", p=P)
            pairs = [(t, min(t + 2, nb)) for t in range(0, nb, 2)]
            next_ag = 0
            for t0, t1 in pairs:
                w = (t1 - t0) * P
                xt = sb.tile([P, kc, 2 * P], f32, tag="xt")
                nc.sync.dma_start(xt[:, :, :w],
                                  xT_r[:, :, t0 * P:t0 * P + w])
                hT = pp_.tile([HID, 2 * P], f32, tag="ps")
                for c in range(kc):
                    nc.tensor.matmul(out=hT[:, :w], lhsT=w1[:, c, :],
                                     rhs=xt[:, c, :w],
                                     start=(c == 0), stop=(c == kc - 1))
                hTs = sb.tile([HID, 2 * P], f32, tag="hTs")
                nc.scalar.copy(hTs[:, :w], hT[:, :w])
                for t in range(t0, t1):
                    off = (t - t0) * P
                    h = pp_.tile([P, HID], f32, tag="ps")
                    nc.tensor.transpose(out=h[:], in_=hTs[:, off:off + P],
                                        identity=ident[:HID, :HID])
                    hp = sb.tile([P, HID], f32, tag="hp")
                    nc.scalar.mul(hp[:], h[:], mul=dinv_sb[:, t:t + 1])
                    nc.sync.dma_start(shard1[t * P:(t + 1) * P, :], hp[:])
                    if stage != 0 and next_ag < n_chunks \
                            and t == ag_after_block[next_ag]:
                        ag_piece(next_ag, shard1, table1)
                        next_ag += 1

            if stage == 0:
                for b in range(nb):
                    t0 = sb.tile([P, HID], f32, tag="cp")
                    nc.sync.dma_start(t0[:], shard1[b * P:(b + 1) * P, :])
                    nc.sync.dma_start(out_d[b * P:(b + 1) * P, :], t0[:])
            if stage == 1:
                for b in range(nb):
                    t0 = sb.tile([P, HID], f32, tag="cp")
                    nc.sync.dma_start(t0[:], table1[b * P:(b + 1) * P, :])
                    nc.sync.dma_start(out_d[b * P:(b + 1) * P, :], t0[:])

            def aggregate(table, layer, gather_only=False, finish=None,
                          piece_done=None):
                colpos = 0
                jpos = 0
                next_pc = 0
                acc = ap_.tile([P, nb * HID], f32, tag=f"acc{layer}",
                               name=f"acc{layer}")
                for ci, (c, w0, qn, blocks) in enumerate(calls):
                    gidx_sb = gxp.tile([P, 8 * qn], i16, tag="gidx")
                    nc.sync.dma_start(
                        gidx_sb[:], gidx_d[:, colpos:colpos + 8 * qn])
                    colpos += 8 * qn
                    npair = sum(len(wj) for _, wj, _, _ in blocks)
                    slots_sb = gxp.tile([P, npair], f32, tag="slots")
                    nc.sync.dma_start(slots_sb[:],
                                      slot_d[:, jpos:jpos + npair])
                    gt = gp.tile([P, qn, HID], f32, tag="g")
                    nc.gpsimd.dma_gather(
                        out_ap=gt[:],
                        in_ap=table[chunk_bases[c]:
                                    chunk_bases[c] + chunk_rows[c], :],
                        idxs_ap=gidx_sb[:], num_idxs=P * qn,
                        num_idxs_reg=P * qn, elem_size=HID,
                        single_packet=False, queue_num=ci % N_QUEUES)
                    if gather_only:
                        jpos += npair
                        continue
                    # one-hot tiles for runs of OHB consecutive pair columns
                    ohs = []
                    for lo in range(0, npair, OHB):
                        g = min(OHB, npair - lo)
                        oh = ohp.tile([P, g, P], f32, tag="oh")
                        nc.vector.tensor_tensor(
                            out=oh[:],
                            in0=iota_sb[:].rearrange("p (g j) -> p g j", g=1)
                                .to_broadcast([P, g, P]),
                            in1=slots_sb[:, lo:lo + g]
                                .rearrange("p (g j) -> p g j", j=1)
                                .to_broadcast([P, g, P]),
                            op=mybir.AluOpType.is_equal)
                        ohs.append(oh)
                    jl = 0
                    for b, wj, accop, fin in blocks:
                        ps = pp_.tile([P, HID], f32, tag="ps")
                        n = len(wj)
                        for i, (w_l, _) in enumerate(wj):
                            nc.tensor.matmul(
                                out=ps[:],
                                lhsT=ohs[jl // OHB][:, jl % OHB, :],
                                rhs=gt[:, w_l, :],
                                start=(i == 0), stop=(i == n - 1))
                            jl += 1
                        a_sl = acc[:, b * HID:(b + 1) * HID]
                        if accop == "copy":
                            nc.scalar.copy(a_sl, ps[:])
                        else:
                            nc.vector.tensor_add(out=a_sl, in0=a_sl,
                                                 in1=ps[:])
                        if fin and finish is not None:
                            finish(b, a_sl)
                    jpos += npair
                    while (next_pc < n_chunks
                           and ci + 1 == piece_call_end[next_pc]):
                        if piece_done is not None:
                            piece_done(next_pc)
                        next_pc += 1

            def self_loop_add(b, a_sl, shard_t, layer):
                s = rp.tile([P, HID], f32, tag=f"sl{layer}",
                            name=f"sl{layer}")
                nc.sync.dma_start(s[:], shard_t[b * P:(b + 1) * P, :])
                tot = rp.tile([P, HID], f32, tag=f"tot{layer}",
                              name=f"tot{layer}")
                nc.vector.tensor_add(out=tot[:], in0=a_sl, in1=s[:])
                return tot

            # ---- layer-1 epilogue: +self-loop, x dinv, +b1, relu, W2, x dinv
            def finish1(b, a_sl):
                tot = self_loop_add(b, a_sl, shard1, 1)
                accb = rp.tile([P, HID], f32, tag="accb", name="accb")
                nc.vector.tensor_scalar(
                    out=accb[:], in0=tot[:], scalar1=dinv_sb[:, b:b + 1],
                    scalar2=None, op0=mybir.AluOpType.mult)
                acc2 = rp.tile([P, HID], f32, tag="acc2", name="acc2")
                nc.vector.tensor_add(out=acc2[:], in0=accb[:], in1=b1_bc[:])
                h2 = sb.tile([P, HID], f32, tag="h2", bufs=6, name="h2")
                nc.scalar.activation(h2[:], acc2[:],
                                     mybir.ActivationFunctionType.Relu)
                h2T = pp_.tile([HID, P], f32, tag="ps", name="h2T")
                nc.tensor.transpose(out=h2T[:], in_=h2[:], identity=ident[:])
                h2Ts = sb.tile([HID, P], f32, tag="h2Ts", bufs=6, name="h2Ts")
                nc.scalar.copy(h2Ts[:], h2T[:])
                gT = pp_.tile([HID, P], f32, tag="ps", name="gT")
                nc.tensor.matmul(out=gT[:], lhsT=w2[:], rhs=h2Ts[:],
                                 start=True, stop=True)
                gTs = sb.tile([HID, P], f32, tag="gTs", bufs=6, name="gTs")
                nc.scalar.copy(gTs[:], gT[:])
                gg = pp_.tile([P, HID], f32, tag="ps", name="gg")
                nc.tensor.transpose(out=gg[:], in_=gTs[:],
                                    identity=ident[:HID, :HID])
                gsb = sb.tile([P, HID], f32, tag="gsb", bufs=6, name="gsb")
                nc.scalar.mul(gsb[:], gg[:], mul=dinv_sb[:, b:b + 1])
                nc.sync.dma_start(shard2[b * P:(b + 1) * P, :], gsb[:])

            def finish3(b, a_sl):
                tot = self_loop_add(b, a_sl, shard1, 1)
                ob = rp.tile([P, HID], f32, tag="ob", name="ob")
                nc.vector.tensor_scalar(
                    out=ob[:], in0=tot[:], scalar1=dinv_sb[:, b:b + 1],
                    scalar2=None, op0=mybir.AluOpType.mult)
                nc.sync.dma_start(out_d[b * P:(b + 1) * P, :], ob[:])

            if stage == 2:
                aggregate(table1, 1, gather_only=True)
            elif stage == 3:
                aggregate(table1, 1, finish=finish3)
            elif stage >= 4:
                aggregate(table1, 1, finish=finish1,
                          piece_done=lambda q: ag_piece(q, shard2, table2))

            # ---- layer-2 epilogue: +self-loop, x dinv, +b2
            def finish2(b, a_sl):
                tot = self_loop_add(b, a_sl, shard2, 2)
                acc4 = rp.tile([P, HID], f32, tag="acc4", name="acc4")
                nc.vector.tensor_scalar(
                    out=acc4[:], in0=tot[:], scalar1=dinv_sb[:, b:b + 1],
                    scalar2=None, op0=mybir.AluOpType.mult)
                osb = sb.tile([P, HID], f32, tag="osb", name="osb")
                nc.vector.tensor_add(out=osb[:], in0=acc4[:], in1=b2_bc[:])
                nc.sync.dma_start(out_d[b * P:(b + 1) * P, :], osb[:])

            if stage >= 4:
                aggregate(table2, 2, finish=finish2)

    nc.compile()
    return nc


# ------------------------------------------------------------------- driver
_CACHE = {}


def _get_nc(in_ch, meta):
    key = (in_ch, meta["shard"], meta["wtot"], meta["n_pairs"], meta["pp"],
           tuple(meta["piece_call_end"]),
           tuple((c, w0, qn, tuple((b, tuple(wj), a, f)
                                   for b, wj, a, f in blocks))
                 for c, w0, qn, blocks in meta["calls"]))
    if key not in _CACHE:
        _CACHE[key] = _build(in_ch, meta)
    return _CACHE[key]


def _in_maps(pre, W1, b1, W2, b2):
    maps = []
    for k in range(N_CORES):
        maps.append({
            "xT": pre["xts"][k],
            "gidx": pre["gidx"][k],
            "slots": pre["slots"][k],
            "iota": pre["iota"],
            "dinv": pre["dinv_cols"][k],
            "W1": W1, "b1": b1, "W2": W2, "b2": b2,
        })
    return maps


def kernel(x, edge_index, W1, b1, W2, b2):
    from concourse.bass_utils import run_bass_kernel_spmd

    x = np.asarray(x, dtype=np.float32)
    W1 = np.ascontiguousarray(np.asarray(W1, dtype=np.float32))
    W2 = np.ascontiguousarray(np.asarray(W2, dtype=np.float32))
    b1 = np.asarray(b1, dtype=np.float32).reshape(1, HID)
    b2 = np.asarray(b2, dtype=np.float32).reshape(1, HID)

    pre = _preprocess(x, edge_index)
    nc = _get_nc(x.shape[1], pre)
    res = run_bass_kernel_spmd(nc, _in_maps(pre, W1, b1, W2, b2),
                               core_ids=list(range(N_CORES)))

    npc = pre["npc"]
    out = np.empty((pre["n_nodes"], HID), dtype=np.float32)
    for k in range(N_CORES):
        out[pre["sorted_nodes"][k]] = res.results[k]["out"][:npc]
    return out


# revision 9
# speedup vs baseline: 1047.6297x; 30.4112x over previous
"""2-layer GCN (PyG GCNConv x2) on 8 Trainium2 NeuronCores via Bass/Tile. v7.

v7 vs v4: the aggregation matmuls run in bf16 — gathered message tiles
are cast f32->bf16 on the ACT engine and one-hot tiles are built directly
in bf16 by the DVE. PE one-hot matmuls drop from 4 to 1 cycle/row and
DVE one-hot builds double in rate. Messages are rounded to bf16 (~3e-3
relative) before the fp32 PSUM accumulation; tolerance is 2e-2.

Sharding: nodes split contiguously across 8 cores (x rows, degree, output);
edges partitioned by destination core so the segment-sum is local; weights
replicated.

v4 vs v3: pipelined collectives.
  * block-cyclic table layout: global row = (pos//pp)*ch + core*pp + pos%pp
    with pp = shard/n_chunks, so table chunk c is the AllGather of each
    core's piece c. Each layer issues n_chunks small AllGathers instead of
    one big one; gather calls on chunk c only wait for AllGather c.
  * piece-major aggregation: dst blocks are processed in n_chunks pieces;
    after piece q of layer-1 finishes, its shard2 piece AllGather is issued
    immediately, overlapping the remaining layer-1 work and letting layer-2
    gathers start while layer 1 is still draining.
  * pads point at row 0 with slot -1 (one-hot selects nothing), so no
    zero-row machinery is needed.

v3 vs v2:
  * dense edge stream per (core, chunk): 128-edge wraps may span dst-block
    boundaries (one-hot slot columns are -1 outside the pair's block), so the
    only gather padding is at chunk tails -> ~28% fewer gather descriptors.
  * self-loop messages are not gathered; the local shard block is added in
    the epilogue instead (saves another ~6% of descriptors).
  * dma_gather calls round-robin over 4 SWDGE queues (parallel Q7
    descriptor generation pipelines), deep gather-tile ring to keep all
    queues busy.

Per core:
  1) transform: hT = W1^T @ xT on PE, rows scaled by deg^-1/2, written
     node-major to a DRAM shard.
  2) AllGather the 64-ch table across the 8 cores.
  3) local segment-sum: real edges sorted by (src-chunk, dst pos). Each
     dma_gather call fetches <=QG 128-edge wraps ([128, qn, 64] tile) from
     one <=32768-row table chunk (int16 index range). Per (wrap, dst block)
     pair, a one-hot selection matrix (DVE is_equal of iota vs the pair's
     slot column; -1 entries select nothing) is matmul-accumulated into
     PSUM: ps_b[slot, ch] += sum_e onehot[e, slot] * msg[e, ch]. Chunk-tail
     pad edges point at a guaranteed-zero pad row.
     Epilogue per block: + local shard block (self-loop), x dinv[dst],
     + bias (+relu, + W2 transform for layer 1).
  4) layer 2 repeats with W2/b2 on a second AllGather'd table.

Host does only sharding-related prep (bucketing/sorting edges, degree
compute, building index/slot streams). All FLOPs and heavy data movement
run on the NeuronCores.
"""

import numpy as np

P = 128
N_CORES = 8
HID = 64
CH_MAX = 32768   # int16 index range per dma_gather call
QG = 64          # max 128-edge wraps per gather call (tile [128, QG, 64])
N_QUEUES = 4     # SWDGE queues; gathers round-robin across Q7 cpu pairs
GBUFS = 5        # gather-tile ring depth (> N_QUEUES keeps all queues busy)
OHB = 8          # one-hot pairs built per DVE op


# ----------------------------------------------------------------- host prep
def _preprocess(x, edge_index, n_cores=N_CORES):
    n_nodes = x.shape[0]
    npc = n_nodes // n_cores
    assert npc * n_cores == n_nodes
    nb = -(-npc // P)
    shard = nb * P
    assert shard > npc, "need pad rows in each shard"
    total = n_cores * shard
    n_chunks = -(-total // CH_MAX)
    assert shard % n_chunks == 0
    pp = shard // n_chunks           # rows per (core, piece)
    ch = n_cores * pp                # table rows per chunk
    chunk_bases = [c * ch for c in range(n_chunks)]
    chunk_rows = [ch] * n_chunks

    src = np.asarray(edge_index[0], dtype=np.int64)
    dst = np.asarray(edge_index[1], dtype=np.int64)
    deg = np.bincount(dst, minlength=n_nodes).astype(np.int64) + 1
    dinv = (1.0 / np.sqrt(deg.astype(np.float64))).astype(np.float32)

    sorted_nodes = []
    row_of = np.empty(n_nodes, dtype=np.int64)
    pos_of = np.empty(n_nodes, dtype=np.int64)
    for k in range(n_cores):
        nodes_k = np.arange(k * npc, (k + 1) * npc)
        order = np.argsort(-deg[nodes_k], kind="stable")
        sn = nodes_k[order]
        sorted_nodes.append(sn)
        pos = np.arange(npc)
        row_of[sn] = (pos // pp) * ch + k * pp + pos % pp
        pos_of[sn] = pos

    # per-core real edges sorted by (src chunk, dst pos); no self-loops
    e_owner = dst // npc
    per_core = []
    E_kc = np.zeros((n_cores, n_chunks), dtype=np.int64)
    for k in range(n_cores):
        m = e_owner == k
        p_e = pos_of[dst[m]]
        sr_e = row_of[src[m]]
        c_e = sr_e // ch
        perm = np.argsort(c_e * shard + p_e, kind="stable")
        p_s, c_s, sr_s = p_e[perm], c_e[perm], sr_e[perm]
        E_kc[k] = np.bincount(c_s, minlength=n_chunks)
        per_core.append((p_s, c_s, sr_s))

    Gc = [int(-(-E_kc[:, c].max() // P)) for c in range(n_chunks)]
    n_pos = int(P * sum(Gc))
    wrap_base = np.concatenate([[0], np.cumsum(Gc)])  # wraps before chunk c

    # (wrap, block) pair union across cores, per chunk
    pair_keys = [set() for _ in range(n_chunks)]
    for k in range(n_cores):
        p_s, c_s, _ = per_core[k]
        for c in range(n_chunks):
            pc = p_s[c_s == c]
            if len(pc) == 0:
                continue
            w_s = np.arange(len(pc)) // P
            b_s = pc // P
            pair_keys[c].update(np.unique(w_s * nb + b_s).tolist())

    # pair index map per chunk: (w, b) -> global pair column j, in program
    # emission order: piece-major (q), then chunk (c), then w-range, then b.
    # A wrap belongs to the piece of its smallest block, so wrap ranges per
    # (q, c) are contiguous.
    chunk_keys = []
    wrap_piece = []
    for c in range(n_chunks):
        keys = np.array(sorted(pair_keys[c]), dtype=np.int64)
        w_all, b_all = keys // nb, keys % nb
        chunk_keys.append((w_all, b_all))
        min_b = np.full(Gc[c], nb, dtype=np.int64)
        np.minimum.at(min_b, w_all, b_all)
        assert (min_b < nb).all(), "empty wrap"
        wrap_piece.append((min_b * P) // pp)

    calls = []       # (c, w0, qn, blocks) ; blocks=[(b, [(w_loc, j)...])...]
    pair_map = [np.full((Gc[c], nb), -1, dtype=np.int64)
                for c in range(n_chunks)]
    occ = {}         # b -> list of (call index, block record)
    piece_call_end = []   # call count after each piece's layer-1 calls
    j = 0
    for q in range(n_chunks):
        for c in range(n_chunks):
            w_all, b_all = chunk_keys[c]
            wsel = np.nonzero(wrap_piece[c] == q)[0]
            if len(wsel) == 0:
                continue
            wlo, whi = int(wsel[0]), int(wsel[-1]) + 1
            for w0 in range(wlo, whi, QG):
                qn = min(QG, whi - w0)
                m = (w_all >= w0) & (w_all < w0 + qn)
                blocks = []
                for b in np.unique(b_all[m]):
                    ws = w_all[m][b_all[m] == b]
                    wj = []
                    for w in ws:
                        pair_map[c][w, b] = j
                        wj.append((int(w - w0), j))
                        j += 1
                    blocks.append([int(b), wj, None, False])
                ci = len(calls)
                for blk in blocks:
                    occ.setdefault(blk[0], []).append((ci, blk))
                calls.append((c, w0, qn, blocks))
        piece_call_end.append(len(calls))
    n_pairs = j

    # acc op (copy on first call-occurrence, add after) + finish flag
    for b, lst in occ.items():
        for i, (ci, blk) in enumerate(lst):
            blk[2] = "copy" if i == 0 else "add"
        lst[-1][1][3] = True
    assert sorted(occ.keys()) == list(range(nb))

    # per-core slot columns and gather index stream
    gidx, slots = [], []
    for k in range(n_cores):
        p_s, c_s, sr_s = per_core[k]
        slots_k = np.full((n_pairs, P), -1.0, dtype=np.float32)
        idx_flat = np.zeros(n_pos, dtype=np.int16)   # pads: row 0, slot -1
        for c in range(n_chunks):
            sel = c_s == c
            pc, sr = p_s[sel], sr_s[sel]
            base = int(wrap_base[c]) * P
            pos = np.arange(len(pc))
            idx_flat[base + pos] = (sr - chunk_bases[c]).astype(np.int16)
            w_s, r_s, b_s = pos // P, pos % P, pc // P
            j_s = pair_map[c][w_s, b_s]
            assert (j_s >= 0).all()
            slots_k[j_s, r_s] = (pc - b_s * P).astype(np.float32)
        segs = []
        for c, w0, qn, _ in calls:
            fl = idx_flat[(wrap_base[c] + w0) * P:
                          (wrap_base[c] + w0 + qn) * P]
            segs.append(np.tile(fl.reshape(-1, 16).T, (8, 1)))
        gidx.append(np.ascontiguousarray(np.concatenate(segs, axis=1)))
        slots.append(np.ascontiguousarray(slots_k.T))

    dinv_cols, xts = [], []
    for k in range(n_cores):
        tmp = np.zeros(shard, dtype=np.float32)
        tmp[:npc] = dinv[sorted_nodes[k]]
        dinv_cols.append(np.ascontiguousarray(tmp.reshape(nb, P).T))
        xt = np.zeros((x.shape[1], shard), dtype=np.float32)
        xt[:, :npc] = np.asarray(x, dtype=np.float32)[sorted_nodes[k]].T
        xts.append(xt)

    iota = np.tile(np.arange(P, dtype=np.float32)[None, :], (P, 1))

    return dict(
        n_nodes=n_nodes, npc=npc, nb=nb, shard=shard, n_chunks=n_chunks,
        pp=pp, chunk_bases=chunk_bases, chunk_rows=chunk_rows, calls=calls,
        piece_call_end=piece_call_end, wtot=gidx[0].shape[1],
        n_pairs=n_pairs, n_pos=n_pos, iota=iota,
        gidx=gidx, slots=slots, dinv_cols=dinv_cols, xts=xts,
        sorted_nodes=sorted_nodes,
    )


# ------------------------------------------------------------- bass program
def _build(in_ch, meta, n_cores=N_CORES, stage=4):
    import concourse.bacc as bacc
    import concourse.tile as tile
    from concourse import mybir
    from concourse.masks import make_identity

    f32 = mybir.dt.float32
    bf16 = mybir.dt.bfloat16
    i16 = mybir.dt.int16
    kc = in_ch // P
    rg = [list(range(n_cores))]
    shard, nb = meta["shard"], meta["nb"]
    calls = meta["calls"]
    chunk_bases, chunk_rows = meta["chunk_bases"], meta["chunk_rows"]
    wtot, n_pairs = meta["wtot"], meta["n_pairs"]
    n_chunks, pp = meta["n_chunks"], meta["pp"]
    piece_call_end = meta["piece_call_end"]
    ch = n_cores * pp

    nc = bacc.Bacc("TRN2", target_bir_lowering=False, debug=False,
                   num_devices=n_cores, num_swdge_queues=N_QUEUES)
    xT_d = nc.dram_tensor("xT", [in_ch, shard], f32, kind="ExternalInput").ap()
    gidx_d = nc.dram_tensor("gidx", [P, wtot], i16, kind="ExternalInput").ap()
    slot_d = nc.dram_tensor("slots", [P, n_pairs], f32,
                            kind="ExternalInput").ap()
    iota_d = nc.dram_tensor("iota", [P, P], f32, kind="ExternalInput").ap()
    dinv_d = nc.dram_tensor("dinv", [P, nb], f32, kind="ExternalInput").ap()
    w1_d = nc.dram_tensor("W1", [in_ch, HID], f32, kind="ExternalInput").ap()
    b1_d = nc.dram_tensor("b1", [1, HID], f32, kind="ExternalInput").ap()
    w2_d = nc.dram_tensor("W2", [HID, HID], f32, kind="ExternalInput").ap()
    b2_d = nc.dram_tensor("b2", [1, HID], f32, kind="ExternalInput").ap()
    out_d = nc.dram_tensor("out", [shard, HID], f32, kind="ExternalOutput").ap()

    with tile.TileContext(nc) as tc:
        with tc.tile_pool(name="const", bufs=1) as cp, \
             tc.tile_pool(name="sb", bufs=3) as sb, \
             tc.tile_pool(name="red", bufs=4) as rp, \
             tc.tile_pool(name="oh", bufs=4) as ohp, \
             tc.tile_pool(name="gat", bufs=GBUFS) as gp, \
             tc.tile_pool(name="gx", bufs=GBUFS) as gxp, \
             tc.tile_pool(name="accp", bufs=1) as ap_, \
             tc.tile_pool(name="ps", bufs=8, space="PSUM") as pp_, \
             tc.tile_pool(name="dram", bufs=1, space="DRAM") as dp:

            w1 = cp.tile([P, kc, HID], f32)
            nc.sync.dma_start(w1[:], w1_d.rearrange("(c p) h -> p c h", p=P))
            w2 = cp.tile([HID, HID], f32)
            nc.sync.dma_start(w2[:], w2_d[:])
            ident = cp.tile([P, P], f32)
            make_identity(nc, ident[:])
            iota_sb = cp.tile([P, P], f32)
            nc.sync.dma_start(iota_sb[:], iota_d[:])
            dinv_sb = cp.tile([P, nb], f32)
            nc.sync.dma_start(dinv_sb[:], dinv_d[:])
            b1_row = cp.tile([1, HID], f32)
            nc.sync.dma_start(b1_row[:], b1_d[:])
            b1_bc = cp.tile([P, HID], f32)
            nc.gpsimd.partition_broadcast(b1_bc[:], b1_row[:])
            b2_row = cp.tile([1, HID], f32)
            nc.sync.dma_start(b2_row[:], b2_d[:])
            b2_bc = cp.tile([P, HID], f32)
            nc.gpsimd.partition_broadcast(b2_bc[:], b2_row[:])

            shard1 = dp.tile([shard, HID], f32)
            table1 = dp.tile([n_cores * shard, HID], f32)
            shard2 = dp.tile([shard, HID], f32)
            table2 = dp.tile([n_cores * shard, HID], f32)

            def ag_piece(q, shard_t, table_t):
                nc.gpsimd.collective_compute(
                    "AllGather", mybir.AluOpType.bypass, replica_groups=rg,
                    ins=[shard_t[q * pp:(q + 1) * pp, :]],
                    outs=[table_t[q * ch:(q + 1) * ch, :]])

            # piece q of the shard is complete once the block holding its
            # last row has been written
            ag_after_block = [-(-pp * (q + 1) // P) - 1
                              for q in range(n_chunks)]

            # ---- layer-1 transform: h1' = dinv * (x @ W1), node-major.
            xT_r = xT_d.rearrange("(c p) n -> p c n", p=P)
            pairs = [(t, min(t + 2, nb)) for t in range(0, nb, 2)]
            next_ag = 0
            for t0, t1 in pairs:
                w = (t1 - t0) * P
                xt = sb.tile([P, kc, 2 * P], f32, tag="xt")
                nc.sync.dma_start(xt[:, :, :w],
                                  xT_r[:, :, t0 * P:t0 * P + w])
                hT = pp_.tile([HID, 2 * P], f32, tag="ps")
                for c in range(kc):
                    nc.tensor.matmul(out=hT[:, :w], lhsT=w1[:, c, :],
                                     rhs=xt[:, c, :w],
                                     start=(c == 0), stop=(c == kc - 1))
                hTs = sb.tile([HID, 2 * P], f32, tag="hTs")
                nc.scalar.copy(hTs[:, :w], hT[:, :w])
                for t in range(t0, t1):
                    off = (t - t0) * P
                    h = pp_.tile([P, HID], f32, tag="ps")
                    nc.tensor.transpose(out=h[:], in_=hTs[:, off:off + P],
                                        identity=ident[:HID, :HID])
                    hp = sb.tile([P, HID], f32, tag="hp")
                    nc.scalar.mul(hp[:], h[:], mul=dinv_sb[:, t:t + 1])
                    nc.sync.dma_start(shard1[t * P:(t + 1) * P, :], hp[:])
                    if stage != 0 and next_ag < n_chunks \
                            and t == ag_after_block[next_ag]:
                        ag_piece(next_ag, shard1, table1)
                        next_ag += 1

            if stage == 0:
                for b in range(nb):
                    t0 = sb.tile([P, HID], f32, tag="cp")
                    nc.sync.dma_start(t0[:], shard1[b * P:(b + 1) * P, :])
                    nc.sync.dma_start(out_d[b * P:(b + 1) * P, :], t0[:])
            if stage == 1:
                for b in range(nb):
                    t0 = sb.tile([P, HID], f32, tag="cp")
                    nc.sync.dma_start(t0[:], table1[b * P:(b + 1) * P, :])
                    nc.sync.dma_start(out_d[b * P:(b + 1) * P, :], t0[:])

            def aggregate(table, layer, gather_only=False, finish=None,
                          piece_done=None):
                colpos = 0
                jpos = 0
                next_pc = 0
                acc = ap_.tile([P, nb * HID], f32, tag=f"acc{layer}",
                               name=f"acc{layer}")
                for ci, (c, w0, qn, blocks) in enumerate(calls):
                    gidx_sb = gxp.tile([P, 8 * qn], i16, tag="gidx")
                    nc.sync.dma_start(
                        gidx_sb[:], gidx_d[:, colpos:colpos + 8 * qn])
                    colpos += 8 * qn
                    npair = sum(len(wj) for _, wj, _, _ in blocks)
                    slots_sb = gxp.tile([P, npair], f32, tag="slots")
                    nc.sync.dma_start(slots_sb[:],
                                      slot_d[:, jpos:jpos + npair])
                    gt = gp.tile([P, qn, HID], f32, tag="g")
                    nc.gpsimd.dma_gather(
                        out_ap=gt[:],
                        in_ap=table[chunk_bases[c]:
                                    chunk_bases[c] + chunk_rows[c], :],
                        idxs_ap=gidx_sb[:], num_idxs=P * qn,
                        num_idxs_reg=P * qn, elem_size=HID,
                        single_packet=False, queue_num=ci % N_QUEUES)
                    if gather_only:
                        jpos += npair
                        continue
                    gtb = gp.tile([P, qn, HID], bf16, tag="gb", bufs=3)
                    nc.scalar.copy(gtb[:], gt[:])
                    # one-hot tiles for runs of OHB consecutive pair columns
                    ohs = []
                    for lo in range(0, npair, OHB):
                        g = min(OHB, npair - lo)
                        oh = ohp.tile([P, g, P], bf16, tag="oh")
                        nc.vector.tensor_tensor(
                            out=oh[:],
                            in0=iota_sb[:].rearrange("p (g j) -> p g j", g=1)
                                .to_broadcast([P, g, P]),
                            in1=slots_sb[:, lo:lo + g]
                                .rearrange("p (g j) -> p g j", j=1)
                                .to_broadcast([P, g, P]),
                            op=mybir.AluOpType.is_equal)
                        ohs.append(oh)
                    jl = 0
                    for b, wj, accop, fin in blocks:
                        ps = pp_.tile([P, HID], f32, tag="ps")
                        n = len(wj)
                        for i, (w_l, _) in enumerate(wj):
                            nc.tensor.matmul(
                                out=ps[:],
                                lhsT=ohs[jl // OHB][:, jl % OHB, :],
                                rhs=gtb[:, w_l, :],
                                start=(i == 0), stop=(i == n - 1))
                            jl += 1
                        a_sl = acc[:, b * HID:(b + 1) * HID]
                        if accop == "copy":
                            nc.scalar.copy(a_sl, ps[:])
                        else:
                            nc.vector.tensor_add(out=a_sl, in0=a_sl,
                                                 in1=ps[:])
                        if fin and finish is not None:
                            finish(b, a_sl)
                    jpos += npair
                    while (next_pc < n_chunks
                           and ci + 1 == piece_call_end[next_pc]):
                        if piece_done is not None:
                            piece_done(next_pc)
                        next_pc += 1

            def self_loop_add(b, a_sl, shard_t, layer):
                s = rp.tile([P, HID], f32, tag=f"sl{layer}",
                            name=f"sl{layer}")
                nc.sync.dma_start(s[:], shard_t[b * P:(b + 1) * P, :])
                tot = rp.tile([P, HID], f32, tag=f"tot{layer}",
                              name=f"tot{layer}")
                nc.vector.tensor_add(out=tot[:], in0=a_sl, in1=s[:])
                return tot

            # ---- layer-1 epilogue: +self-loop, x dinv, +b1, relu, W2, x dinv
            def finish1(b, a_sl):
                tot = self_loop_add(b, a_sl, shard1, 1)
                accb = rp.tile([P, HID], f32, tag="accb", name="accb")
                nc.vector.tensor_scalar(
                    out=accb[:], in0=tot[:], scalar1=dinv_sb[:, b:b + 1],
                    scalar2=None, op0=mybir.AluOpType.mult)
                acc2 = rp.tile([P, HID], f32, tag="acc2", name="acc2")
                nc.vector.tensor_add(out=acc2[:], in0=accb[:], in1=b1_bc[:])
                h2 = sb.tile([P, HID], f32, tag="h2", bufs=6, name="h2")
                nc.scalar.activation(h2[:], acc2[:],
                                     mybir.ActivationFunctionType.Relu)
                h2T = pp_.tile([HID, P], f32, tag="ps", name="h2T")
                nc.tensor.transpose(out=h2T[:], in_=h2[:], identity=ident[:])
                h2Ts = sb.tile([HID, P], f32, tag="h2Ts", bufs=6, name="h2Ts")
                nc.scalar.copy(h2Ts[:], h2T[:])
                gT = pp_.tile([HID, P], f32, tag="ps", name="gT")
                nc.tensor.matmul(out=gT[:], lhsT=w2[:], rhs=h2Ts[:],
                                 start=True, stop=True)
                gTs = sb.tile([HID, P], f32, tag="gTs", bufs=6, name="gTs")
                nc.scalar.copy(gTs[:], gT[:])
                gg = pp_.tile([P, HID], f32, tag="ps", name="gg")
                nc.tensor.transpose(out=gg[:], in_=gTs[:],
                                    identity=ident[:HID, :HID])
                gsb = sb.tile([P, HID], f32, tag="gsb", bufs=6, name="gsb")
                nc.scalar.mul(gsb[:], gg[:], mul=dinv_sb[:, b:b + 1])
                nc.sync.dma_start(shard2[b * P:(b + 1) * P, :], gsb[:])

            def finish3(b, a_sl):
                tot = self_loop_add(b, a_sl, shard1, 1)
                ob = rp.tile([P, HID], f32, tag="ob", name="ob")
                nc.vector.tensor_scalar(
                    out=ob[:], in0=tot[:], scalar1=dinv_sb[:, b:b + 1],
                    scalar2=None, op0=mybir.AluOpType.mult)
                nc.sync.dma_start(out_d[b * P:(b + 1) * P, :], ob[:])

            if stage == 2:
                aggregate(table1, 1, gather_only=True)
            elif stage == 3:
                aggregate(table1, 1, finish=finish3)
            elif stage >= 4:
                aggregate(table1, 1, finish=finish1,
                          piece_done=lambda q: ag_piece(q, shard2, table2))

            # ---- layer-2 epilogue: +self-loop, x dinv, +b2
            def finish2(b, a_sl):
                tot = self_loop_add(b, a_sl, shard2, 2)
                acc4 = rp.tile([P, HID], f32, tag="acc4", name="acc4")
                nc.vector.tensor_scalar(
                    out=acc4[:], in0=tot[:], scalar1=dinv_sb[:, b:b + 1],
                    scalar2=None, op0=mybir.AluOpType.mult)
                osb = sb.tile([P, HID], f32, tag="osb", name="osb")
                nc.vector.tensor_add(out=osb[:], in0=acc4[:], in1=b2_bc[:])
                nc.sync.dma_start(out_d[b * P:(b + 1) * P, :], osb[:])

            if stage >= 4:
                aggregate(table2, 2, finish=finish2)

    nc.compile()
    return nc


# ------------------------------------------------------------------- driver
_CACHE = {}


def _get_nc(in_ch, meta):
    key = (in_ch, meta["shard"], meta["wtot"], meta["n_pairs"], meta["pp"],
           tuple(meta["piece_call_end"]),
           tuple((c, w0, qn, tuple((b, tuple(wj), a, f)
                                   for b, wj, a, f in blocks))
                 for c, w0, qn, blocks in meta["calls"]))
    if key not in _CACHE:
        _CACHE[key] = _build(in_ch, meta)
    return _CACHE[key]


def _in_maps(pre, W1, b1, W2, b2):
    maps = []
    for k in range(N_CORES):
        maps.append({
            "xT": pre["xts"][k],
            "gidx": pre["gidx"][k],
            "slots": pre["slots"][k],
            "iota": pre["iota"],
            "dinv": pre["dinv_cols"][k],
            "W1": W1, "b1": b1, "W2": W2, "b2": b2,
        })
    return maps


def kernel(x, edge_index, W1, b1, W2, b2):
    from concourse.bass_utils import run_bass_kernel_spmd

    x = np.asarray(x, dtype=np.float32)
    W1 = np.ascontiguousarray(np.asarray(W1, dtype=np.float32))
    W2 = np.ascontiguousarray(np.asarray(W2, dtype=np.float32))
    b1 = np.asarray(b1, dtype=np.float32).reshape(1, HID)
    b2 = np.asarray(b2, dtype=np.float32).reshape(1, HID)

    pre = _preprocess(x, edge_index)
    nc = _get_nc(x.shape[1], pre)
    res = run_bass_kernel_spmd(nc, _in_maps(pre, W1, b1, W2, b2),
                               core_ids=list(range(N_CORES)))

    npc = pre["npc"]
    out = np.empty((pre["n_nodes"], HID), dtype=np.float32)
    for k in range(N_CORES):
        out[pre["sorted_nodes"][k]] = res.results[k]["out"][:npc]
    return out
